# revision 77
# baseline (speedup 1.0000x reference)
"""RBF-kernel attention (nn_Attention_76081050682051) on 8 TRN2 NeuronCores.

Self-contained Bass/Tile kernel. `kernel(**inputs)` takes the FULL unsharded
inputs of reference.setup_inputs() and returns the FULL [4, 2048, 256] f32
output.

Sharding (B x tensor-parallel heads): core c -> batch b = c//2, heads
[4*(c%2), 4*(c%2)+4); pairwise AllReduce ([0,1],[2,3],[4,5],[6,7]) combines
the two half-head partial outputs of each batch after the W_o projection.

Device math:
  LayerNorm per-partition via bn_stats/bn_aggr + DVE-reciprocal/Newton
  rsqrt; xnT blocks via PE transposes (f32r identity = 1.5 cycles/row).
  K'/Q' projections run in f32r (11-bit mantissa, full PE rate) with
  sqrt(2*gamma)*ln_scale*2^WSH folded into W_q/W_k on the host; the exact
  f32r psum feeds BOTH the k2/q2 row sums (which need ~11-bit precision:
  exponent errors are amplified ~50x) AND an fp8e4m3 hi/lo split.
  The S x S score matmul then runs as THREE fp8 DoubleRow matmuls per
  [128,512] tile -- (kh,qh),(kl,qh),(kh,ql), each contracting both 128-e
  chunks in its two weight slots at 0.5 cycles/row -- i.e. hybrid-3-term
  fp8 with bf16-grade accuracy at 0.75x the f32r cycle cost.
  The k2/q2 bias/scale quarters come straight off the sq tiles as tiny
  [128,2] column matmuls (sq slice stationary, ones moving -- matmul
  moving free size must be >= 2 per the ISA checker), landing k2/q2
  with s/t already on partitions: no [1,512] row matmuls, no DRAM
  round-trip, no 4x128 transpose.
  scoresT[t, s] = exp(2^-2*WSH * qk - k2/2) via one ACT op per tile
  (per-partition bias, scale compensates the host 2^WSH folding); the
  exp(-q2/2) factor is applied after W_o as a per-partition scale.
  The V projection is FOLDED INTO W_o on the host: out_h =
  (scores.T @ xn) @ (Wv_h @ Wo_h), so the main loop accumulates
  gT = xn.T-weighted scores with the raw xnu tiles (kept alive, s-major)
  as stationary -- the whole V stage (projection matmuls + f32r copies)
  disappears.  gT accumulates over t in f32r PSUM (scores need >8-bit
  mantissa -- pure-fp8 scores measure 2.4%% output error vs the 2%% gate,
  and the output Frobenius mass concentrates in ~50 rows whose score
  columns are single-element dominated, so fp8's 6%% element rounding
  never averages out -- this matmul stays f32r); M = Wv@Wo runs on gT
  column slices; partial outputs AllReduce within each batch pair, in
  quarters so the stores overlap the last head's compute.
  Emission is software-pipelined: the next head's projection work and
  the previous s-block's W_o units are interleaved one-per-tile between
  main-loop score tiles (PE's in-order 4-deep wait queue head-of-line
  blocks on any burst whose psum ring or inputs are busy); ops(tt-SKEW)
  is emitted before stps(tt) so ready work is never stuck behind a
  slot wait.  Engine balance: DVE carries LN/psum copies/hi splits, Pool
  (no PSUM access on gpsimd) carries the SBUF-side lo splits and squares,
  ACT runs the exps (+ head-0 prep copies while it is still exp-free).
  Front scheduling (cost-model): DMAs issue ~625ns apart on one HWDGE
  ring and transfers serialize on one DMA-engines device (~22.5 B/ns,
  +900ns completion-sem), so x loads go first (block 0 in quarters,
  later blocks in halves) and head-0 weights after; LN Newton chains are
  split per half-block.  ACT's exec queue is depth 0 and DVE/Pool wait
  queues are 4-deep, so emission order keeps parked copies out of the
  front (every attempt to interleave head-0 proj with the LN chains
  measured SLOWER -- the front is vector-queue-bound).
"""
import sys
sys.path.insert(0, '/opt/trn_rl_repo')
import numpy as np
from concourse import bass, bacc, tile, mybir, masks
from concourse.bass_utils import run_bass_kernel_spmd

F32 = mybir.dt.float32
F32R = mybir.dt.float32r
FP8 = mybir.dt.float8e4
AF = mybir.ActivationFunctionType
OP = mybir.AluOpType
PM = mybir.MatmulPerfMode

# W_q/W_k are folded x2^WSH on the host so the fp8 hi/lo split of K'/Q'
# stays clear of e4m3's subnormal floor; the exp compensates via
# scale=2^-2*WSH (and the k2/q2 quarters via their -0.5 scales).
WSH = 6
SCL = float(2.0 ** (-2 * WSH))

B, S, E, H = 4, 2048, 256, 8
HL = 4          # heads per core
EC = 2          # e chunks of 128
SB = 4          # s blocks of 512
ST = 16         # s/t tiles of 128
N_CORES = 8
EPS = 1e-5

NO_COLL = False
N_HEADS_BUILD = HL


def build_kernel(R=1, debug=False):
    nc = bacc.Bacc("TRN2", target_bir_lowering=False, debug=False,
                   num_devices=N_CORES)

    x_ext = nc.declare_dram_parameter("x", [S, E], F32, isOutput=False)
    w_ext = {}
    for wname in ("wq", "wk", "wo"):
        # host pre-lays out as [head, partition, ec*e] so the per-head load
        # is one contiguous 2-D DMA (HWDGE, no SWDGE descriptor generation)
        w_ext[wname] = nc.declare_dram_parameter(wname, [HL, 128, EC * E], F32,
                                                 isOutput=False)
    out_ext = nc.declare_dram_parameter("out", [S, E], F32, isOutput=True)
    dbg_ext = {}
    if debug:
        dbg_ext['xn'] = nc.declare_dram_parameter("dbg_xn", [E, S], F32, isOutput=True)
        dbg_ext['qt'] = nc.declare_dram_parameter("dbg_qt", [E, S], F32, isOutput=True)
        dbg_ext['v'] = nc.declare_dram_parameter("dbg_v", [128, ST * E], F32, isOutput=True)
        dbg_ext['q2'] = nc.declare_dram_parameter("dbg_q2", [128, ST], F32, isOutput=True)
        dbg_ext['part'] = nc.declare_dram_parameter("dbg_part", [128, ST * E], F32, isOutput=True)

    with tile.TileContext(nc) as tc:
        with tc.tile_pool(name="sb", bufs=1) as sb, \
             tc.tile_pool(name="sbt", bufs=1) as sbt, \
             tc.tile_pool(name="ps", bufs=1, space="PSUM") as ps, \
             tc.tile_pool(name="dram", bufs=1, space="DRAM") as dram:

            # ---------- constants ----------
            # [128,2]: matmul moving free size must be >= 2 (ISA check), so
            # the k2/q2 column matmuls write duplicated column pairs
            ones_col32 = sb.tile([128, 2], F32, name="ones_col32")
            nc.any.memset(ones_col32[:], 1.0)
            ones_col = sb.tile([128, 2], F32R, name="ones_col")
            nc.vector.tensor_copy(ones_col[:], ones_col32[:])
            ident16 = sb.tile([16, 16], F32, name="ident16")
            masks.make_identity(nc, ident16[:])
            ident128 = sb.tile([128, 128], F32, name="ident128")
            masks.make_identity(nc, ident128[:])
            # neuronxcc rejects mixed 32/8-bit matmul inputs; f32r identity
            # still runs the f32r-data transpose at 1.5 cycles/row (vs 2.0)
            ident128_8 = sb.tile([128, 128], F32R, name="ident128r")
            nc.vector.tensor_copy(ident128_8[:], ident128[:])

            # ---------- load x blocks first (sync queue) ----------
            # sbk 0 in single-tile quarters: LN stats j0 starts ~1us earlier
            # (HWDGE issues are serialized at ~625ns each, so granularity on
            # the FIRST block shortens the critical path; later blocks batch)
            # Serial-DMA-device order tuned to the two critical paths (first
            # projection needs wk+xn(0); the main loop additionally needs
            # xn(3), whose LN chain starts only after x3h1): wk, x block 0
            # quarters, wq, x blocks 1-3 halves, wo.
            w_early = {}

            def load_weight(wname):
                wtmp = sbt.tile([128, EC * E], F32, name="wtmp",
                                tag="wtmp", bufs=3)
                nc.sync.dma_start(wtmp[:], w_ext[wname][0])
                w_early[wname] = wtmp

            xu_tiles = []
            for sbk in range(SB):
                xu = sbt.tile([128, 4 * E], F32, name="xu", tag="xu", bufs=4)
                # halves for later blocks: 728ns transfers pipeline cleanly
                # against the 625ns HWDGE issues, landing block 3 ~1.5us
                # earlier than one 1456ns transfer at the queue tail
                nch = 4 if sbk == 0 else 2
                for hh in range(nch):
                    step = 4 // nch
                    t0, t1 = hh * step, (hh + 1) * step
                    nc.sync.dma_start(
                        xu[:, t0 * E:t1 * E].rearrange("p (t e) -> p t e",
                                                       t=t1 - t0),
                        x_ext[sbk * 512 + t0 * 128:sbk * 512 + t1 * 128, :]
                        .rearrange("(t p) e -> p t e", p=128))
                xu_tiles.append(xu)
            for wname in ("wk", "wq", "wo"):
                load_weight(wname)

            pools = dict(sb=sb, sbt=sbt, ps=ps, dram=dram)
            _build_body(nc, tc, pools, xu_tiles, w_ext, ones_col, ident16,
                        ident128_8, out_ext, dbg_ext, w_early)

    nc.compile()
    return nc


def _build_body(nc, tc, pools, xu_tiles, w_ext, ones_col, ident16, ident128,
                out_ext, dbg_ext, w_early=None):
    sb, sbt, ps, dram = pools['sb'], pools['sbt'], pools['ps'], pools['dram']

    def mm_pool(shape, tag="mm", bufs=2):
        return ps.tile(shape, F32, name=tag, tag=tag, bufs=bufs)

    SL = [slice(i * 512, (i + 1) * 512) for i in range(SB)]

    # ============ LayerNorm (per-partition stats, per s-block chains) ============
    # one [128, EC, 512] tile per block: both ec transposes of a j-tile
    # share one psum tile and ONE psum->SBUF copy (16 copies instead of 32,
    # ~3us off the saturated front DVE/ACT)
    xnb = {}
    for sbk in range(SB):
        xnb[sbk] = sb.tile([128, EC, 512], F32R, name=f"xn_{sbk}")

    from contextlib import nullcontext

    # LN stats land per s-block as its x chunk arrives; ONE batched
    # eps/rsqrt-Newton chain covers all 16 row-tiles (the serial Newton is
    # chain-latency, not throughput, so batching it shortens the front)
    mvall = sb.tile([128, 4 * SB, 2], F32, name="mvall")
    inva = sb.tile([128, 4 * SB], F32, name="inva")

    def emit_ln_stats(sbk, j0=0, j1=4, st6_tile=None):
        _prio = tc.high_priority() if sbk == 0 else nullcontext()
        _prio.__enter__()
        xu = xu_tiles[sbk]
        st6 = st6_tile if st6_tile is not None else \
            sbt.tile([128, 4, 6], F32, name="st6", tag="st6", bufs=2)
        for j in range(j0, j1):
            nc.vector.bn_stats(st6[:, j], xu[:, j * E:(j + 1) * E])
            nc.vector.bn_aggr(mvall[:, sbk * 4 + j], st6[:, j])
        _prio.__exit__(None, None, None)
        return st6

    def emit_ln_newton(i0, n):
        """Batched eps/rsqrt-Newton for row-tiles [i0, i0+n)."""
        va = sbt.tile([128, n], F32, name="va", tag="va", bufs=2)
        vb = sbt.tile([128, n], F32, name="vb", tag="vb", bufs=2)
        iva = inva[:, i0:i0 + n]
        nc.vector.tensor_scalar_add(vb[:], mvall[:, i0:i0 + n, 1], EPS)
        # rsqrt(v) without ACT: v is concentrated near 1 (var of 256-sample
        # LN), so y0 = (1 + 1/v)/2 ~ 1/sqrt(v) to 2nd order; 3 Newton steps
        # take worst-case |v-1| ~ 0.5 to < 1e-6 relative.
        with nc.allow_low_precision("newton-polished below"):
            nc.vector.reciprocal(iva, vb[:])
        nc.vector.tensor_scalar(iva, iva, 0.5, 0.5, OP.mult, OP.add)
        for _ in range(2):
            nc.vector.tensor_mul(va[:], iva, iva)
            nc.vector.tensor_mul(va[:], va[:], vb[:])
            nc.vector.tensor_scalar(va[:], va[:], -0.5, 1.5, OP.mult, OP.add)
            nc.vector.tensor_mul(iva, iva, va[:])

    xnu_tiles = {}

    def emit_ln_xnu(sbk, j0=0, j1=4):
        # vector side only; the PE transposes are emitted separately so the
        # in-order PE queue can interleave them with head-0's projections
        xu = xu_tiles[sbk]
        for j in range(j0, j1):
            i = sbk * 4 + j
            xnu = sbt.tile([128, E], F32R, name="xnu", tag="xnu", bufs=16)
            eng = nc.vector if j % 2 == 0 else nc.gpsimd
            eng.tensor_scalar(xnu[:], xu[:, j * E:(j + 1) * E],
                              mvall[:, i, 0:1], inva[:, i:i + 1],
                              OP.subtract, OP.mult)
            xnu_tiles[sbk, j] = xnu

    def emit_ln_transposes(sbk, j0=0, j1=4):
        # xnu tiles stay alive: they are the lhsT of the g = scores.T @ xn
        # accumulation in the main loop (V projection folded into W_o).
        # Both psum copies go to DVE: ACT's exec queue is depth 0, so a
        # head-0 p32 copy parked there would stall later blocks' transposes
        for j in range(j0, j1):
            xnu = xnu_tiles[sbk, j]
            pt = ps.tile([128, 2, 128], F32R, name="mmv", tag="mmv", bufs=2)
            for ec in range(EC):
                nc.tensor.transpose(pt[:, ec], xnu[:, ec * 128:(ec + 1) * 128],
                                    ident128[:])
            dst = xnb[sbk][:, :, j * 128:(j + 1) * 128]
            if j % 2 == 0:
                nc.scalar.copy(dst, pt[:])
            else:
                nc.vector.tensor_copy(dst, pt[:])

    def emit_ln(sbk):
        if sbk == 0:
            # j0/j1 chain first: their x quarters land earliest and the
            # serial Newton latency (~1.5us) overlaps j2/j3's stats + DMA;
            # high priority keeps later blocks' stats from stealing DVE
            # slots between the serial Newton links
            with tc.high_priority():
                st6 = emit_ln_stats(0, 0, 2)
                emit_ln_newton(0, 2)
                emit_ln_stats(0, 2, 4, st6_tile=st6)
                emit_ln_xnu(0, 0, 2)
                emit_ln_newton(2, 2)
                emit_ln_xnu(0, 2, 4)
        else:
            # per-half chains: the j0/j1 Newton runs while j2/j3's x half is
            # still in the DMA queue
            st6 = emit_ln_stats(sbk, 0, 2)
            emit_ln_newton(sbk * 4, 2)
            emit_ln_stats(sbk, 2, 4, st6_tile=st6)
            emit_ln_xnu(sbk, 0, 2)
            emit_ln_newton(sbk * 4 + 2, 2)
            emit_ln_xnu(sbk, 2, 4)

    # ============ per-head attention ============
    acc = sb.tile([128, ST * E], F32, name="acc")
    if N_HEADS_BUILD == 0:
        nc.any.memset(acc[:], 0.0)

    # four quarter-bounce tiles: each AllReduce quarter fires as soon as the
    # last head's W_o finishes its s-block, overlapping remaining compute
    bounce_in = [dram.tile([S // 4, E], F32, name=f"bounce_in{i}",
                           tag=f"bin{i}", bufs=1) for i in range(4)]
    bounce_view = [b.rearrange("(t p) e -> p t e", p=128) for b in bounce_in]

    # Per-head state; emission is software-pipelined across heads so head
    # h+1's (DVE-heavy) projection copies overlap head h's (PE/ACT-heavy)
    # main loop.  Slot grants within a pool tag are FIFO in emission order,
    # so interleaved emission is what actually enables the overlap.
    st_h = {}

    def proj_pp(h, wname, sbk, ft, sqs):
        """One 128-row chunk of a K'/Q' projection: f32r matmul pair into
        PSUM, split to fp8 hi/lo for the DoubleRow score matmuls, square the
        exact psum for the k2/q2 row quarter."""
        s = st_h[h]
        hi_tag = "kth" if wname == "wk" else "qth"
        lo_tag = "ktl" if wname == "wk" else "qtl"
        if ft == 0:
            s[hi_tag][sbk] = sbt.tile([128, EC, 512], FP8, name=hi_tag,
                                      tag=hi_tag, bufs=8)
            s[lo_tag][sbk] = sbt.tile([128, EC, 512], FP8, name=lo_tag,
                                      tag=lo_tag, bufs=8)
        hi, lo = s[hi_tag][sbk], s[lo_tag][sbk]
        wr = s['w'][wname]
        # pp frees after its single p32 reader (~1us); steady-state pps are
        # ~3 tiles apart so one mm bank suffices, and head 0's pps borrow
        # the (still idle) stps ring -- freeing a bank for stps bufs=3
        pp = mm_pool([128, 512], tag="stps", bufs=3) if h == 0 else \
            mm_pool([128, 512], bufs=1)
        for ec in range(EC):
            o = ec * E + ft * 128
            nc.tensor.matmul(pp[:], wr[:, o:o + 128], xnb[sbk][:, ec, :],
                             start=(ec == 0), stop=(ec == EC - 1))
        # single psum reader (fast pp-ring release); hi/lo/sq derive from
        # the SBUF copy, where the (psum-incapable) Pool engine can help
        p32 = sbt.tile([128, 512], F32R, name="p32", tag="p32", bufs=8)
        if h == 0:
            nc.scalar.copy(p32[:], pp[:])         # ACT is exp-free early
        else:
            nc.vector.tensor_copy(p32[:], pp[:])
        nc.vector.tensor_copy(hi[:, ft, :], p32[:])
        nc.gpsimd.tensor_tensor(lo[:, ft, :], p32[:].bitcast(F32),
                                hi[:, ft, :], OP.subtract)
        sq = sbt.tile([128, 512], F32R, name="sqc", tag="sqc", bufs=6)
        if h == 0:
            # front: Pool carries the lo splits; DVE has slack for squares
            nc.vector.tensor_mul(sq[:], p32[:].bitcast(F32), p32[:].bitcast(F32))
        else:
            nc.gpsimd.tensor_tensor(sq[:], p32[:].bitcast(F32),
                                    p32[:].bitcast(F32), OP.mult)
        sqs.append(sq)

    def cols_direct(h, sqs, i, is_exp, no_act=False):
        """k2/q2 quarter DIRECTLY as per-partition columns: 8 tiny [128,1]
        matmuls with the sq tile as stationary and ones as moving (~4ns each
        vs the old [1,512] row matmuls at 218ns + DRAM round-trip +
        transpose).  Partitions of the psum are the s/t index, exactly the
        layout the ACT bias / eq scale needs."""
        cps = ps.tile([128, 8], F32, name="cps", tag="mmv", bufs=2)
        for tj in range(4):
            for ft in range(EC):
                nc.tensor.matmul(cps[:, 2 * tj:2 * tj + 2],
                                 sqs[2 * i + ft][:, tj * 128:(tj + 1) * 128],
                                 ones_col[:],
                                 start=(ft == 0), stop=(ft == EC - 1))
        colsq = sbt.tile([128, 4], F32, name="colsq",
                         tag="biasq" if not is_exp else "eq2q", bufs=12)
        if is_exp:
            nc.scalar.activation(colsq[:], cps[:, ::2], AF.Exp, scale=-0.5 * SCL)
        elif h == 0 and not no_act:
            nc.scalar.activation(colsq[:], cps[:, ::2], AF.Identity,
                                 scale=-0.5 * SCL)
        else:
            nc.vector.tensor_scalar_mul(colsq[:], cps[:, ::2], -0.5 * SCL)
        return colsq

    def proj_fillers(h, sbk):
        """Per-pp emission closures for one s-block's K+Q projections, to be
        interleaved between main-loop tiles (keeps the pp ring from
        head-of-line-blocking the in-order PE queue)."""
        sqs = []
        fs = [lambda w=w, ft=ft: proj_pp(h, w, sbk, ft, sqs)
              for w in ("wk", "wq") for ft in range(EC)]

        def tail_k(no_act=False):
            s = st_h[h]
            s['biasq'][sbk] = cols_direct(h, sqs, 0, is_exp=False,
                                          no_act=no_act)

        def tail_q():
            # q2 is only consumed by this s-block's W_o units a whole s-block
            # later; emitting it as its own (late) filler keeps the tiny
            # matmuls from waiting on the Pool-side squares in the in-order
            # PE queue
            s = st_h[h]
            s['eq2q'][sbk] = cols_direct(h, sqs, 1, is_exp=True)
        fs.append(tail_k)
        fs.append(tail_q)
        return fs

    def emit_proj(h, sbk):
        for f in proj_fillers(h, sbk):
            f()

    def emit_bias(h):
        pass

    def main_tiles(h, sbk, fillers=()):
        """Generator: one main-loop tile (stps triplet + exp + lagged ops)
        per iteration, so the caller can interleave other emission."""
        s = st_h[h]
        kth, ktl, qth, qtl = s['kth'], s['ktl'], s['qth'], s['qtl']
        biasq = s['biasq']
        qh = qth[sbk]
        ql = qtl[sbk]

        fillers = list(fillers)
        # spread fillers evenly over the loop (a dry stretch re-couples the
        # PE/ACT cadence; a dense burst head-of-line blocks the PE queue)
        nf = len(fillers)
        pops = {1 + (i * (ST + 2)) // nf: i for i in range(nf)} if nf else {}
        ops = [ps.tile([128, 512], F32, name="ovps", tag=f"ovps{ft}", bufs=1)
               for ft in range(EC)]
        sc_q = {}
        SKEW = 4
        for tt in range(ST + SKEW):
            if tt:
                yield
            while fillers and pops.get(tt) is not None and \
                    pops[tt] >= nf - len(fillers):
                fillers.pop(0)()
                break
            # ops of tt-SKEW go BEFORE stps of tt: when stps waits for its
            # psum slot (ACT release), this ready work isn't stuck behind it
            # in the in-order PE queue
            if tt >= SKEW:
                # g accumulation: lhsT = raw xn rows of t-tile (V projection
                # is folded into the host-side M = Wv @ Wo), rhs = scores
                pv_tt = tt - SKEW
                sc_prev = sc_q.pop(pv_tt)
                xnu = xnu_tiles[divmod(pv_tt, 4)]
                for ft in range(EC):
                    nc.tensor.matmul(ops[ft][:],
                                     xnu[:, ft * 128:(ft + 1) * 128],
                                     sc_prev[:],
                                     start=(pv_tt == 0), stop=(pv_tt == ST - 1))
            if tt < ST:
                tb, tj = divmod(tt, 4)
                kh = kth[tb][:, :, tj * 128:(tj + 1) * 128]
                kl = ktl[tb][:, :, tj * 128:(tj + 1) * 128]
                stps = mm_pool([128, 512], tag="stps", bufs=3)
                # hybrid3: (kh+kl)(qh+ql) - kl*ql; each DoubleRow inst
                # contracts both 128-e chunks at 0.5 cycles/row
                nc.tensor.matmul(stps[:], kh, qh[:], start=True, stop=False,
                                 perf_mode=PM.DoubleRow)
                nc.tensor.matmul(stps[:], kl, qh[:], start=False, stop=False,
                                 perf_mode=PM.DoubleRow)
                nc.tensor.matmul(stps[:], kh, ql[:], start=False, stop=True,
                                 perf_mode=PM.DoubleRow)
                sc = sbt.tile([128, 512], F32R, name="sc", tag="sc", bufs=6)
                nc.scalar.activation(sc[:], stps[:], AF.Exp,
                                     bias=biasq[tb][:, tj:tj + 1], scale=SCL)
                sc_q[tt] = sc
        for f in fillers:
            f()
        for ft in range(EC):
            o = sbt.tile([128, 512], F32R, name="outT", tag="outT", bufs=8)
            if ft == 0 and h > 0:
                nc.scalar.copy(o[:], ops[ft][:])
            else:
                nc.vector.tensor_copy(o[:], ops[ft][:])
            s['outT'][ft, sbk] = o

    def emit_main(h, sbk, fillers=()):
        for _ in main_tiles(h, sbk, fillers):
            pass

    def wo_unit(h, st):
        s = st_h[h]
        wo = s['w']['wo']
        sbk, j = divmod(st, 4)
        wops = mm_pool([128, E], tag="mmv", bufs=2)
        for ft in range(EC):
            nc.tensor.matmul(wops[:], s['outT'][ft, sbk][:, j * 128:(j + 1) * 128],
                             wo[:, ft * E:(ft + 1) * E],
                             start=(ft == 0), stop=(ft == EC - 1))
        asl = acc[:, st * E:(st + 1) * E]
        eqcol = s['eq2q'][sbk][:, j:j + 1]
        if h == 0:
            nc.vector.tensor_scalar(asl, wops[:], eqcol, None, OP.mult)
        else:
            nc.vector.scalar_tensor_tensor(asl, wops[:], eqcol,
                                           asl, OP.mult, OP.add)
        # (wops reads PSUM so the acc op stays off Pool)

    def wo_fillers(h, sbk):
        fs = [lambda st=st: wo_unit(h, st)
              for st in range(sbk * 4, sbk * 4 + 4)]
        if h == N_HEADS_BUILD - 1:
            def bounce(sbk=sbk, half=None):
                # one batched 3-D DMA per s-block (4 tiles), not 4 setups;
                # the final s-block goes in halves so its out-copy overlaps
                t0, t1 = (0, 4) if half is None else (2 * half, 2 * half + 2)
                nc.sync.dma_start(
                    bounce_view[sbk][:, t0:t1, :],
                    acc[:, (sbk * 4 + t0) * E:(sbk * 4 + t1) * E]
                    .rearrange("p (t e) -> p t e", e=E))
            if sbk == SB - 1:
                fs.insert(2, lambda: bounce(half=0))
                fs.append(lambda: bounce(half=1))
            else:
                fs.append(bounce)
        return fs

    def emit_wo(h, sbk):
        for f in wo_fillers(h, sbk):
            f()

    def finish_head_state(h, wname):
        wtmp = w_early[wname] if (h == 0 and w_early and wname in w_early) \
            else None
        if wtmp is None:
            wtmp = sbt.tile([128, EC * E], F32, name="wtmp", tag="wtmp",
                            bufs=3)
            nc.sync.dma_start(wtmp[:], w_ext[wname][h])
        wr = sbt.tile([128, EC * E], F32R, name=f"w_{wname}",
                      tag=f"w_{wname}", bufs=2)
        if h == 0 and wname in ("wk", "wq"):
            nc.scalar.copy(wr[:], wtmp[:])
        else:
            nc.vector.tensor_copy(wr[:], wtmp[:])
        st_h[h]['w'][wname] = wr

    def new_head_state(h, skip=()):
        st_h[h] = dict(w={}, kth={}, ktl={}, qth={}, qtl={}, outT={},
                       biasq={}, eq2q={})
        for wname in ("wk", "wq", "wo"):
            if wname not in skip:
                finish_head_state(h, wname)

    # Per-block [LN vec, transposes], with head-0 projections LAGGING the
    # transposes by one block: the in-order PE queue runs block k's
    # projections while block k+1's LN/DMA chain is still in flight, and
    # ACT's depth-0 queue never sees a parked p32 copy ahead of the next
    # block's transpose copies
    if N_HEADS_BUILD > 0:
        for sbk in range(SB):
            emit_ln(sbk)
            emit_ln_transposes(sbk)
        new_head_state(0)
        for sbk in range(SB):
            emit_proj(0, sbk)

    for h in range(N_HEADS_BUILD):
        nxt = h + 1
        if nxt < N_HEADS_BUILD:
            new_head_state(nxt)
        for sbk in range(SB):
            # Interleave between this head's main-loop tiles: (a) the W_o
            # units of the previous s-block (their outT copies have had a
            # whole s-block to land), (b) the next head's projections.  The
            # in-order PE queue then never sees a burst of matmuls whose
            # psum ring or inputs are busy.
            # one projection block + one V block per stretch keeps the
            # DVE/Pool split work evenly loaded instead of bursty; round-
            # robin so same-psum-ring allocations are never adjacent
            lanes = []
            late = []
            if False and h == 0 and sbk == 0 and SB > 2:
                # proj(0,2)/(0,3) first (their K splits gate tiles 8-15);
                # biasq on DVE and the eq2q exps dead last so neither parks
                # in ACT's depth-0 queue ahead of this block's own exps.
                # proj(1,0) goes late too: its pps wait on head-1 weights
                # and must not crowd the 4-deep PE wait queue early.
                # sequential (not round-robin): block 2's bias quarter must
                # be emitted before tile 8's exp, block 3's before tile 12
                seq = []
                for psbk in range(2, SB):
                    fs = proj_fillers(0, psbk)
                    fs[-2] = (lambda f=fs[-2]: f(no_act=True))
                    late.append(fs.pop())
                    seq += fs
                lanes.append(seq)
                if nxt < N_HEADS_BUILD:
                    late = proj_fillers(nxt, sbk) + late
            elif nxt < N_HEADS_BUILD:
                lanes += [proj_fillers(nxt, sbk)]
            if sbk > 0:
                lanes.insert(1, wo_fillers(h, sbk - 1))
            elif h > 0:
                lanes.insert(1, wo_fillers(h - 1, 3))
            fillers = []
            while any(lanes):
                for ln in lanes:
                    if ln:
                        fillers.append(ln.pop(0))
            fillers += late
            emit_main(h, sbk, fillers)
        if h == N_HEADS_BUILD - 1:
            emit_wo(h, 3)

        if h > 0:
            st_h.pop(h - 1, None)

    if dbg_ext:
        nc.sync.dma_start(dbg_ext['part'][:], acc[:])

    if N_HEADS_BUILD == 0:
        for q in range(4):
            nc.sync.dma_start(
                bounce_view[q][:, :, :],
                acc[:, q * 4 * E:(q + 1) * 4 * E]
                .rearrange("p (t e) -> p t e", e=E))

    # ============ AllReduce over batch pair + store ============
    # four quarters; the last quarter in halves so its store overlaps the
    # second half's bounce write
    QS = S // 4
    pieces = [(q * QS, QS) for q in range(3)] + \
             [(3 * QS, QS // 2), (3 * QS + QS // 2, QS // 2)]
    for pi, (r0, n) in enumerate(pieces):
        osl = out_ext[r0:r0 + n, :]
        q, b0 = divmod(r0, QS)
        if NO_COLL:
            nc.sync.dma_start(osl, bounce_in[q][b0:b0 + n, :])
        else:
            bo = dram.tile([n, E], F32, name=f"bounce_out{pi}",
                           tag=f"bout{pi}", bufs=1)
            nc.gpsimd.collective_compute(
                "AllReduce", OP.add,
                replica_groups=[[0, 1], [2, 3], [4, 5], [6, 7]],
                ins=[bounce_in[q][b0:b0 + n, :].opt()],
                outs=[bo.opt()],
            )
            nc.sync.dma_start(osl, bo[:, :])


# ================= host side =================

def prep_inputs(x, ln_scale, W_q, W_k, W_v, W_o, gamma):
    """Build per-core input maps."""
    x = np.asarray(x, np.float32)
    ln_scale = np.asarray(ln_scale, np.float32)
    W_q = np.asarray(W_q, np.float32)
    W_k = np.asarray(W_k, np.float32)
    W_v = np.asarray(W_v, np.float32)
    W_o = np.asarray(W_o, np.float32)
    gamma = np.asarray(gamma, np.float32).reshape(H)

    in_maps = []
    for c in range(N_CORES):
        b = c // 2
        h0 = HL * (c % 2)
        hs = list(range(h0, h0 + HL))
        g = gamma[hs]
        # 2^WSH folded into W_q/W_k (undone by the exp's scale=2^-2*WSH) so
        # the fp8 hi/lo split of K'/Q' sits well inside e4m3's normal range
        s2g = (np.sqrt(2.0 * g) * (2.0 ** WSH)).astype(np.float32)
        wq = (W_q[hs] * ln_scale[None, :, None] * s2g[:, None, None])
        wk = (W_k[hs] * ln_scale[None, :, None] * s2g[:, None, None])

        def _lay(w):   # [HL, E_in(=EC*128), E] -> [HL, 128, EC*E]
            return np.ascontiguousarray(
                w.reshape(HL, EC, 128, E).transpose(0, 2, 1, 3).reshape(HL, 128, EC * E))
        wq = _lay(wq)
        wk = _lay(wk)
        # V folded into W_o: out_h = (scores @ xn) @ (Wv_h @ Wo_h)
        wo = _lay(np.stack([
            (W_v[h] * ln_scale[:, None]).astype(np.float64)
            @ W_o[:, 256 * h:256 * (h + 1)].T.astype(np.float64)
            for h in hs]).astype(np.float32))
        in_maps.append({
            "x": np.ascontiguousarray(x[b]),
            "wq": np.ascontiguousarray(wq),
            "wk": np.ascontiguousarray(wk),
            "wo": np.ascontiguousarray(wo),
        })
    return in_maps


def assemble_output(results):
    out = np.empty((B, S, E), np.float32)
    for b in range(B):
        out[b] = results[2 * b]["out"]
    return out


_NC_CACHE = {}


def _get_nc():
    if 'nc' not in _NC_CACHE:
        _NC_CACHE['nc'] = build_kernel(R=1, debug=False)
    return _NC_CACHE['nc']


def kernel(x, e=None, p=None, ln_scale=None, W_q=None, W_k=None, W_v=None,
           W_o=None, gamma=None, **_unused):
    """Full-input entry point. e and p are unused by the reference network
    (use_ppe=False config); they are accepted and ignored."""
    in_maps = prep_inputs(x, ln_scale, W_q, W_k, W_v, W_o, gamma)
    nc = _get_nc()
    res = run_bass_kernel_spmd(nc, in_maps, core_ids=list(range(N_CORES)))
    return assemble_output(res.results)



# revision 79
# speedup vs baseline: 1.0013x; 1.0013x over previous
"""RBF-kernel attention (nn_Attention_76081050682051) on 8 TRN2 NeuronCores.

Self-contained Bass/Tile kernel. `kernel(**inputs)` takes the FULL unsharded
inputs of reference.setup_inputs() and returns the FULL [4, 2048, 256] f32
output.

Sharding (B x tensor-parallel heads): core c -> batch b = c//2, heads
[4*(c%2), 4*(c%2)+4); pairwise AllReduce ([0,1],[2,3],[4,5],[6,7]) combines
the two half-head partial outputs of each batch after the W_o projection.

Device math:
  LayerNorm per-partition via bn_stats/bn_aggr + DVE-reciprocal/Newton
  rsqrt; xnT blocks via PE transposes (f32r identity = 1.5 cycles/row).
  K'/Q' projections run in f32r (11-bit mantissa, full PE rate) with
  sqrt(2*gamma)*ln_scale*2^WSH folded into W_q/W_k on the host; the exact
  f32r psum feeds BOTH the k2/q2 row sums (which need ~11-bit precision:
  exponent errors are amplified ~50x) AND an fp8e4m3 hi/lo split.
  The S x S score matmul then runs as THREE fp8 DoubleRow matmuls per
  [128,512] tile -- (kh,qh),(kl,qh),(kh,ql), each contracting both 128-e
  chunks in its two weight slots at 0.5 cycles/row -- i.e. hybrid-3-term
  fp8 with bf16-grade accuracy at 0.75x the f32r cycle cost.
  The k2/q2 bias/scale quarters come straight off the sq tiles as tiny
  [128,2] column matmuls (sq slice stationary, ones moving -- matmul
  moving free size must be >= 2 per the ISA checker), landing k2/q2
  with s/t already on partitions: no [1,512] row matmuls, no DRAM
  round-trip, no 4x128 transpose.
  scoresT[t, s] = exp(2^-2*WSH * qk - k2/2) via one ACT op per tile
  (per-partition bias, scale compensates the host 2^WSH folding); the
  exp(-q2/2) factor is applied after W_o as a per-partition scale.
  The V projection is FOLDED INTO W_o on the host: out_h =
  (scores.T @ xn) @ (Wv_h @ Wo_h), so the main loop accumulates
  gT = xn.T-weighted scores with the raw xnu tiles (kept alive, s-major)
  as stationary -- the whole V stage (projection matmuls + f32r copies)
  disappears.  gT accumulates over t in f32r PSUM (scores need >8-bit
  mantissa -- pure-fp8 scores measure 2.4%% output error vs the 2%% gate,
  and the output Frobenius mass concentrates in ~50 rows whose score
  columns are single-element dominated, so fp8's 6%% element rounding
  never averages out -- this matmul stays f32r); M = Wv@Wo runs on gT
  column slices; partial outputs AllReduce within each batch pair, in
  quarters so the stores overlap the last head's compute.
  Emission is software-pipelined: the next head's projection work and
  the previous s-block's W_o units are interleaved one-per-tile between
  main-loop score tiles (PE's in-order 4-deep wait queue head-of-line
  blocks on any burst whose psum ring or inputs are busy); ops(tt-SKEW)
  is emitted before stps(tt) so ready work is never stuck behind a
  slot wait.  Engine balance: DVE carries LN/psum copies/hi splits, Pool
  (no PSUM access on gpsimd) carries the SBUF-side lo splits and squares,
  ACT runs the exps (+ head-0 prep copies while it is still exp-free).
  Front scheduling (cost-model): DMAs issue ~625ns apart on one HWDGE
  ring and transfers serialize on one DMA-engines device (~22.5 B/ns,
  +900ns completion-sem), so x loads go first (block 0 in quarters,
  later blocks in halves) and head-0 weights after; LN Newton chains are
  split per half-block.  ACT's exec queue is depth 0 and DVE/Pool wait
  queues are 4-deep, so emission order keeps parked copies out of the
  front (every attempt to interleave head-0 proj with the LN chains
  measured SLOWER -- the front is vector-queue-bound).
"""
import sys
sys.path.insert(0, '/opt/trn_rl_repo')
import numpy as np
from concourse import bass, bacc, tile, mybir, masks
from concourse.bass_utils import run_bass_kernel_spmd

F32 = mybir.dt.float32
F32R = mybir.dt.float32r
FP8 = mybir.dt.float8e4
AF = mybir.ActivationFunctionType
OP = mybir.AluOpType
PM = mybir.MatmulPerfMode

# W_q/W_k are folded x2^WSH on the host so the fp8 hi/lo split of K'/Q'
# stays clear of e4m3's subnormal floor; the exp compensates via
# scale=2^-2*WSH (and the k2/q2 quarters via their -0.5 scales).
WSH = 6
SCL = float(2.0 ** (-2 * WSH))

B, S, E, H = 4, 2048, 256, 8
HL = 4          # heads per core
EC = 2          # e chunks of 128
SB = 4          # s blocks of 512
ST = 16         # s/t tiles of 128
N_CORES = 8
EPS = 1e-5

NO_COLL = False
N_HEADS_BUILD = HL


def build_kernel(R=1, debug=False):
    nc = bacc.Bacc("TRN2", target_bir_lowering=False, debug=False,
                   num_devices=N_CORES)

    x_ext = nc.declare_dram_parameter("x", [S, E], F32, isOutput=False)
    w_ext = {}
    for wname in ("wq", "wk", "wo"):
        # host pre-lays out as [head, partition, ec*e] so the per-head load
        # is one contiguous 2-D DMA (HWDGE, no SWDGE descriptor generation)
        w_ext[wname] = nc.declare_dram_parameter(wname, [HL, 128, EC * E], F32,
                                                 isOutput=False)
    out_ext = nc.declare_dram_parameter("out", [S, E], F32, isOutput=True)
    dbg_ext = {}
    if debug:
        dbg_ext['xn'] = nc.declare_dram_parameter("dbg_xn", [E, S], F32, isOutput=True)
        dbg_ext['qt'] = nc.declare_dram_parameter("dbg_qt", [E, S], F32, isOutput=True)
        dbg_ext['v'] = nc.declare_dram_parameter("dbg_v", [128, ST * E], F32, isOutput=True)
        dbg_ext['q2'] = nc.declare_dram_parameter("dbg_q2", [128, ST], F32, isOutput=True)
        dbg_ext['part'] = nc.declare_dram_parameter("dbg_part", [128, ST * E], F32, isOutput=True)

    with tile.TileContext(nc) as tc:
        with tc.tile_pool(name="sb", bufs=1) as sb, \
             tc.tile_pool(name="sbt", bufs=1) as sbt, \
             tc.tile_pool(name="ps", bufs=1, space="PSUM") as ps, \
             tc.tile_pool(name="dram", bufs=1, space="DRAM") as dram:

            # ---------- constants ----------
            # [128,2]: matmul moving free size must be >= 2 (ISA check), so
            # the k2/q2 column matmuls write duplicated column pairs
            ones_col32 = sb.tile([128, 2], F32, name="ones_col32")
            nc.any.memset(ones_col32[:], 1.0)
            ones_col = sb.tile([128, 2], F32R, name="ones_col")
            nc.vector.tensor_copy(ones_col[:], ones_col32[:])
            ident16 = sb.tile([16, 16], F32, name="ident16")
            masks.make_identity(nc, ident16[:])
            ident128 = sb.tile([128, 128], F32, name="ident128")
            masks.make_identity(nc, ident128[:])
            # neuronxcc rejects mixed 32/8-bit matmul inputs; f32r identity
            # still runs the f32r-data transpose at 1.5 cycles/row (vs 2.0)
            ident128_8 = sb.tile([128, 128], F32R, name="ident128r")
            nc.vector.tensor_copy(ident128_8[:], ident128[:])

            # ---------- load x blocks first (sync queue) ----------
            # sbk 0 in single-tile quarters: LN stats j0 starts ~1us earlier
            # (HWDGE issues are serialized at ~625ns each, so granularity on
            # the FIRST block shortens the critical path; later blocks batch)
            # Serial-DMA-device order tuned to the two critical paths (first
            # projection needs wk+xn(0); the main loop additionally needs
            # xn(3), whose LN chain starts only after x3h1): wk, x block 0
            # quarters, wq, x blocks 1-3 halves, wo.
            w_early = {}

            def load_weight(wname):
                wtmp = sbt.tile([128, EC * E], F32, name="wtmp",
                                tag="wtmp", bufs=3)
                nc.sync.dma_start(wtmp[:], w_ext[wname][0])
                w_early[wname] = wtmp

            xu_tiles = []
            for sbk in range(SB):
                xu = sbt.tile([128, 4 * E], F32, name="xu", tag="xu", bufs=4)
                # halves for later blocks: 728ns transfers pipeline cleanly
                # against the 625ns HWDGE issues, landing block 3 ~1.5us
                # earlier than one 1456ns transfer at the queue tail
                nch = 2
                for hh in range(nch):
                    step = 4 // nch
                    t0, t1 = hh * step, (hh + 1) * step
                    nc.sync.dma_start(
                        xu[:, t0 * E:t1 * E].rearrange("p (t e) -> p t e",
                                                       t=t1 - t0),
                        x_ext[sbk * 512 + t0 * 128:sbk * 512 + t1 * 128, :]
                        .rearrange("(t p) e -> p t e", p=128))
                xu_tiles.append(xu)
            for wname in ("wk", "wq", "wo"):
                load_weight(wname)

            pools = dict(sb=sb, sbt=sbt, ps=ps, dram=dram)
            _build_body(nc, tc, pools, xu_tiles, w_ext, ones_col, ident16,
                        ident128_8, out_ext, dbg_ext, w_early)

    nc.compile()
    return nc


def _build_body(nc, tc, pools, xu_tiles, w_ext, ones_col, ident16, ident128,
                out_ext, dbg_ext, w_early=None):
    sb, sbt, ps, dram = pools['sb'], pools['sbt'], pools['ps'], pools['dram']

    def mm_pool(shape, tag="mm", bufs=2):
        return ps.tile(shape, F32, name=tag, tag=tag, bufs=bufs)

    SL = [slice(i * 512, (i + 1) * 512) for i in range(SB)]

    # ============ LayerNorm (per-partition stats, per s-block chains) ============
    # one [128, EC, 512] tile per block: both ec transposes of a j-tile
    # share one psum tile and ONE psum->SBUF copy (16 copies instead of 32,
    # ~3us off the saturated front DVE/ACT)
    xnb = {}
    for sbk in range(SB):
        xnb[sbk] = sb.tile([128, EC, 512], F32R, name=f"xn_{sbk}")

    from contextlib import nullcontext

    # LN stats land per s-block as its x chunk arrives; ONE batched
    # eps/rsqrt-Newton chain covers all 16 row-tiles (the serial Newton is
    # chain-latency, not throughput, so batching it shortens the front)
    mvall = sb.tile([128, 4 * SB, 2], F32, name="mvall")
    inva = sb.tile([128, 4 * SB], F32, name="inva")

    def emit_ln_stats(sbk, j0=0, j1=4, st6_tile=None):
        _prio = tc.high_priority() if sbk == 0 else nullcontext()
        _prio.__enter__()
        xu = xu_tiles[sbk]
        st6 = st6_tile if st6_tile is not None else \
            sbt.tile([128, 4, 6], F32, name="st6", tag="st6", bufs=2)
        for j in range(j0, j1):
            nc.vector.bn_stats(st6[:, j], xu[:, j * E:(j + 1) * E])
            nc.vector.bn_aggr(mvall[:, sbk * 4 + j], st6[:, j])
        _prio.__exit__(None, None, None)
        return st6

    def emit_ln_newton(i0, n):
        """Batched eps/rsqrt-Newton for row-tiles [i0, i0+n)."""
        va = sbt.tile([128, n], F32, name="va", tag="va", bufs=2)
        vb = sbt.tile([128, n], F32, name="vb", tag="vb", bufs=2)
        iva = inva[:, i0:i0 + n]
        nc.vector.tensor_scalar_add(vb[:], mvall[:, i0:i0 + n, 1], EPS)
        # rsqrt(v) without ACT: v is concentrated near 1 (var of 256-sample
        # LN), so y0 = (1 + 1/v)/2 ~ 1/sqrt(v) to 2nd order; 3 Newton steps
        # take worst-case |v-1| ~ 0.5 to < 1e-6 relative.
        with nc.allow_low_precision("newton-polished below"):
            nc.vector.reciprocal(iva, vb[:])
        nc.vector.tensor_scalar(iva, iva, 0.5, 0.5, OP.mult, OP.add)
        for _ in range(2):
            nc.vector.tensor_mul(va[:], iva, iva)
            nc.vector.tensor_mul(va[:], va[:], vb[:])
            nc.vector.tensor_scalar(va[:], va[:], -0.5, 1.5, OP.mult, OP.add)
            nc.vector.tensor_mul(iva, iva, va[:])

    xnu_tiles = {}

    def emit_ln_xnu(sbk, j0=0, j1=4):
        # vector side only; the PE transposes are emitted separately so the
        # in-order PE queue can interleave them with head-0's projections
        xu = xu_tiles[sbk]
        for j in range(j0, j1):
            i = sbk * 4 + j
            xnu = sbt.tile([128, E], F32R, name="xnu", tag="xnu", bufs=16)
            eng = nc.vector if j % 2 == 0 else nc.gpsimd
            eng.tensor_scalar(xnu[:], xu[:, j * E:(j + 1) * E],
                              mvall[:, i, 0:1], inva[:, i:i + 1],
                              OP.subtract, OP.mult)
            xnu_tiles[sbk, j] = xnu

    def emit_ln_transposes(sbk, j0=0, j1=4):
        # xnu tiles stay alive: they are the lhsT of the g = scores.T @ xn
        # accumulation in the main loop (V projection folded into W_o).
        # Both psum copies go to DVE: ACT's exec queue is depth 0, so a
        # head-0 p32 copy parked there would stall later blocks' transposes
        for j in range(j0, j1):
            xnu = xnu_tiles[sbk, j]
            pt = ps.tile([128, 2, 128], F32R, name="mmv", tag="mmv", bufs=2)
            for ec in range(EC):
                nc.tensor.transpose(pt[:, ec], xnu[:, ec * 128:(ec + 1) * 128],
                                    ident128[:])
            dst = xnb[sbk][:, :, j * 128:(j + 1) * 128]
            if j % 2 == 0:
                nc.scalar.copy(dst, pt[:])
            else:
                nc.vector.tensor_copy(dst, pt[:])

    def emit_ln(sbk):
        if sbk == 0:
            # j0/j1 chain first: their x quarters land earliest and the
            # serial Newton latency (~1.5us) overlaps j2/j3's stats + DMA;
            # high priority keeps later blocks' stats from stealing DVE
            # slots between the serial Newton links
            with tc.high_priority():
                st6 = emit_ln_stats(0, 0, 2)
                emit_ln_newton(0, 2)
                emit_ln_stats(0, 2, 4, st6_tile=st6)
                emit_ln_xnu(0, 0, 2)
                emit_ln_newton(2, 2)
                emit_ln_xnu(0, 2, 4)
        else:
            # per-half chains: the j0/j1 Newton runs while j2/j3's x half is
            # still in the DMA queue
            st6 = emit_ln_stats(sbk, 0, 2)
            emit_ln_newton(sbk * 4, 2)
            emit_ln_stats(sbk, 2, 4, st6_tile=st6)
            emit_ln_xnu(sbk, 0, 2)
            emit_ln_newton(sbk * 4 + 2, 2)
            emit_ln_xnu(sbk, 2, 4)

    # ============ per-head attention ============
    acc = sb.tile([128, ST * E], F32, name="acc")
    if N_HEADS_BUILD == 0:
        nc.any.memset(acc[:], 0.0)

    # four quarter-bounce tiles: each AllReduce quarter fires as soon as the
    # last head's W_o finishes its s-block, overlapping remaining compute
    bounce_in = [dram.tile([S // 4, E], F32, name=f"bounce_in{i}",
                           tag=f"bin{i}", bufs=1) for i in range(4)]
    bounce_view = [b.rearrange("(t p) e -> p t e", p=128) for b in bounce_in]

    # Per-head state; emission is software-pipelined across heads so head
    # h+1's (DVE-heavy) projection copies overlap head h's (PE/ACT-heavy)
    # main loop.  Slot grants within a pool tag are FIFO in emission order,
    # so interleaved emission is what actually enables the overlap.
    st_h = {}

    def proj_pp(h, wname, sbk, ft, sqs):
        """One 128-row chunk of a K'/Q' projection: f32r matmul pair into
        PSUM, split to fp8 hi/lo for the DoubleRow score matmuls, square the
        exact psum for the k2/q2 row quarter."""
        s = st_h[h]
        hi_tag = "kth" if wname == "wk" else "qth"
        lo_tag = "ktl" if wname == "wk" else "qtl"
        if ft == 0:
            s[hi_tag][sbk] = sbt.tile([128, EC, 512], FP8, name=hi_tag,
                                      tag=hi_tag, bufs=8)
            s[lo_tag][sbk] = sbt.tile([128, EC, 512], FP8, name=lo_tag,
                                      tag=lo_tag, bufs=8)
        hi, lo = s[hi_tag][sbk], s[lo_tag][sbk]
        wr = s['w'][wname]
        # pp frees after its single p32 reader (~1us); steady-state pps are
        # ~3 tiles apart so one mm bank suffices, and head 0's pps borrow
        # the (still idle) stps ring -- freeing a bank for stps bufs=3
        pp = mm_pool([128, 512], tag="stps", bufs=3) if h == 0 else \
            mm_pool([128, 512], bufs=1)
        for ec in range(EC):
            o = ec * E + ft * 128
            nc.tensor.matmul(pp[:], wr[:, o:o + 128], xnb[sbk][:, ec, :],
                             start=(ec == 0), stop=(ec == EC - 1))
        # single psum reader (fast pp-ring release); hi/lo/sq derive from
        # the SBUF copy, where the (psum-incapable) Pool engine can help
        p32 = sbt.tile([128, 512], F32R, name="p32", tag="p32", bufs=8)
        if h == 0:
            nc.scalar.copy(p32[:], pp[:])         # ACT is exp-free early
        else:
            nc.vector.tensor_copy(p32[:], pp[:])
        nc.vector.tensor_copy(hi[:, ft, :], p32[:])
        nc.gpsimd.tensor_tensor(lo[:, ft, :], p32[:].bitcast(F32),
                                hi[:, ft, :], OP.subtract)
        sq = sbt.tile([128, 512], F32R, name="sqc", tag="sqc", bufs=6)
        if h == 0:
            # front: Pool carries the lo splits; DVE has slack for squares
            nc.vector.tensor_mul(sq[:], p32[:].bitcast(F32), p32[:].bitcast(F32))
        else:
            nc.gpsimd.tensor_tensor(sq[:], p32[:].bitcast(F32),
                                    p32[:].bitcast(F32), OP.mult)
        sqs.append(sq)

    def cols_direct(h, sqs, i, is_exp, no_act=False):
        """k2/q2 quarter DIRECTLY as per-partition columns: 8 tiny [128,1]
        matmuls with the sq tile as stationary and ones as moving (~4ns each
        vs the old [1,512] row matmuls at 218ns + DRAM round-trip +
        transpose).  Partitions of the psum are the s/t index, exactly the
        layout the ACT bias / eq scale needs."""
        cps = ps.tile([128, 8], F32, name="cps", tag="mmv", bufs=2)
        for tj in range(4):
            for ft in range(EC):
                nc.tensor.matmul(cps[:, 2 * tj:2 * tj + 2],
                                 sqs[2 * i + ft][:, tj * 128:(tj + 1) * 128],
                                 ones_col[:],
                                 start=(ft == 0), stop=(ft == EC - 1))
        colsq = sbt.tile([128, 4], F32, name="colsq",
                         tag="biasq" if not is_exp else "eq2q", bufs=12)
        if is_exp:
            nc.scalar.activation(colsq[:], cps[:, ::2], AF.Exp, scale=-0.5 * SCL)
        elif h == 0 and not no_act:
            nc.scalar.activation(colsq[:], cps[:, ::2], AF.Identity,
                                 scale=-0.5 * SCL)
        else:
            nc.vector.tensor_scalar_mul(colsq[:], cps[:, ::2], -0.5 * SCL)
        return colsq

    def proj_fillers(h, sbk):
        """Per-pp emission closures for one s-block's K+Q projections, to be
        interleaved between main-loop tiles (keeps the pp ring from
        head-of-line-blocking the in-order PE queue)."""
        sqs = []
        fs = [lambda w=w, ft=ft: proj_pp(h, w, sbk, ft, sqs)
              for w in ("wk", "wq") for ft in range(EC)]

        def tail_k(no_act=False):
            s = st_h[h]
            s['biasq'][sbk] = cols_direct(h, sqs, 0, is_exp=False,
                                          no_act=no_act)

        def tail_q():
            # q2 is only consumed by this s-block's W_o units a whole s-block
            # later; emitting it as its own (late) filler keeps the tiny
            # matmuls from waiting on the Pool-side squares in the in-order
            # PE queue
            s = st_h[h]
            s['eq2q'][sbk] = cols_direct(h, sqs, 1, is_exp=True)
        fs.append(tail_k)
        fs.append(tail_q)
        return fs

    def emit_proj(h, sbk):
        for f in proj_fillers(h, sbk):
            f()

    def emit_bias(h):
        pass

    def main_tiles(h, sbk, fillers=()):
        """Generator: one main-loop tile (stps triplet + exp + lagged ops)
        per iteration, so the caller can interleave other emission."""
        s = st_h[h]
        kth, ktl, qth, qtl = s['kth'], s['ktl'], s['qth'], s['qtl']
        biasq = s['biasq']
        qh = qth[sbk]
        ql = qtl[sbk]

        fillers = list(fillers)
        # spread fillers evenly over the loop (a dry stretch re-couples the
        # PE/ACT cadence; a dense burst head-of-line blocks the PE queue)
        nf = len(fillers)
        pops = {1 + (i * (ST + 2)) // nf: i for i in range(nf)} if nf else {}
        ops = [ps.tile([128, 512], F32, name="ovps", tag=f"ovps{ft}", bufs=1)
               for ft in range(EC)]
        sc_q = {}
        SKEW = 4
        for tt in range(ST + SKEW):
            if tt:
                yield
            while fillers and pops.get(tt) is not None and \
                    pops[tt] >= nf - len(fillers):
                fillers.pop(0)()
                break
            # ops of tt-SKEW go BEFORE stps of tt: when stps waits for its
            # psum slot (ACT release), this ready work isn't stuck behind it
            # in the in-order PE queue
            if tt >= SKEW:
                # g accumulation: lhsT = raw xn rows of t-tile (V projection
                # is folded into the host-side M = Wv @ Wo), rhs = scores
                pv_tt = tt - SKEW
                sc_prev = sc_q.pop(pv_tt)
                xnu = xnu_tiles[divmod(pv_tt, 4)]
                for ft in range(EC):
                    nc.tensor.matmul(ops[ft][:],
                                     xnu[:, ft * 128:(ft + 1) * 128],
                                     sc_prev[:],
                                     start=(pv_tt == 0), stop=(pv_tt == ST - 1))
            if tt < ST:
                tb, tj = divmod(tt, 4)
                kh = kth[tb][:, :, tj * 128:(tj + 1) * 128]
                kl = ktl[tb][:, :, tj * 128:(tj + 1) * 128]
                stps = mm_pool([128, 512], tag="stps", bufs=3)
                # hybrid3: (kh+kl)(qh+ql) - kl*ql; each DoubleRow inst
                # contracts both 128-e chunks at 0.5 cycles/row
                nc.tensor.matmul(stps[:], kh, qh[:], start=True, stop=False,
                                 perf_mode=PM.DoubleRow)
                nc.tensor.matmul(stps[:], kl, qh[:], start=False, stop=False,
                                 perf_mode=PM.DoubleRow)
                nc.tensor.matmul(stps[:], kh, ql[:], start=False, stop=True,
                                 perf_mode=PM.DoubleRow)
                sc = sbt.tile([128, 512], F32R, name="sc", tag="sc", bufs=6)
                nc.scalar.activation(sc[:], stps[:], AF.Exp,
                                     bias=biasq[tb][:, tj:tj + 1], scale=SCL)
                sc_q[tt] = sc
        for f in fillers:
            f()
        for ft in range(EC):
            o = sbt.tile([128, 512], F32R, name="outT", tag="outT", bufs=8)
            if ft == 0 and h > 0:
                nc.scalar.copy(o[:], ops[ft][:])
            else:
                nc.vector.tensor_copy(o[:], ops[ft][:])
            s['outT'][ft, sbk] = o

    def emit_main(h, sbk, fillers=()):
        for _ in main_tiles(h, sbk, fillers):
            pass

    def wo_unit(h, st):
        s = st_h[h]
        wo = s['w']['wo']
        sbk, j = divmod(st, 4)
        wops = mm_pool([128, E], tag="mmv", bufs=2)
        for ft in range(EC):
            nc.tensor.matmul(wops[:], s['outT'][ft, sbk][:, j * 128:(j + 1) * 128],
                             wo[:, ft * E:(ft + 1) * E],
                             start=(ft == 0), stop=(ft == EC - 1))
        asl = acc[:, st * E:(st + 1) * E]
        eqcol = s['eq2q'][sbk][:, j:j + 1]
        if h == 0:
            nc.vector.tensor_scalar(asl, wops[:], eqcol, None, OP.mult)
        else:
            nc.vector.scalar_tensor_tensor(asl, wops[:], eqcol,
                                           asl, OP.mult, OP.add)
        # (wops reads PSUM so the acc op stays off Pool)

    def wo_fillers(h, sbk):
        fs = [lambda st=st: wo_unit(h, st)
              for st in range(sbk * 4, sbk * 4 + 4)]
        if h == N_HEADS_BUILD - 1:
            def bounce(sbk=sbk, half=None):
                # one batched 3-D DMA per s-block (4 tiles), not 4 setups;
                # the final s-block goes in halves so its out-copy overlaps
                t0, t1 = (0, 4) if half is None else (2 * half, 2 * half + 2)
                nc.sync.dma_start(
                    bounce_view[sbk][:, t0:t1, :],
                    acc[:, (sbk * 4 + t0) * E:(sbk * 4 + t1) * E]
                    .rearrange("p (t e) -> p t e", e=E))
            if sbk == SB - 1:
                fs.insert(2, lambda: bounce(half=0))
                fs.append(lambda: bounce(half=1))
            else:
                fs.append(bounce)
        return fs

    def emit_wo(h, sbk):
        for f in wo_fillers(h, sbk):
            f()

    def finish_head_state(h, wname):
        wtmp = w_early[wname] if (h == 0 and w_early and wname in w_early) \
            else None
        if wtmp is None:
            wtmp = sbt.tile([128, EC * E], F32, name="wtmp", tag="wtmp",
                            bufs=3)
            nc.sync.dma_start(wtmp[:], w_ext[wname][h])
        wr = sbt.tile([128, EC * E], F32R, name=f"w_{wname}",
                      tag=f"w_{wname}", bufs=2)
        if h == 0 and wname in ("wk", "wq"):
            nc.scalar.copy(wr[:], wtmp[:])
        else:
            nc.vector.tensor_copy(wr[:], wtmp[:])
        st_h[h]['w'][wname] = wr

    def new_head_state(h, skip=()):
        st_h[h] = dict(w={}, kth={}, ktl={}, qth={}, qtl={}, outT={},
                       biasq={}, eq2q={})
        for wname in ("wk", "wq", "wo"):
            if wname not in skip:
                finish_head_state(h, wname)

    # Per-block [LN vec, transposes], with head-0 projections LAGGING the
    # transposes by one block: the in-order PE queue runs block k's
    # projections while block k+1's LN/DMA chain is still in flight, and
    # ACT's depth-0 queue never sees a parked p32 copy ahead of the next
    # block's transpose copies
    if N_HEADS_BUILD > 0:
        for sbk in range(SB):
            emit_ln(sbk)
            emit_ln_transposes(sbk)
        new_head_state(0)
        for sbk in range(SB):
            emit_proj(0, sbk)

    for h in range(N_HEADS_BUILD):
        nxt = h + 1
        if nxt < N_HEADS_BUILD:
            new_head_state(nxt)
        for sbk in range(SB):
            # Interleave between this head's main-loop tiles: (a) the W_o
            # units of the previous s-block (their outT copies have had a
            # whole s-block to land), (b) the next head's projections.  The
            # in-order PE queue then never sees a burst of matmuls whose
            # psum ring or inputs are busy.
            # one projection block + one V block per stretch keeps the
            # DVE/Pool split work evenly loaded instead of bursty; round-
            # robin so same-psum-ring allocations are never adjacent
            lanes = []
            late = []
            if False and h == 0 and sbk == 0 and SB > 2:
                # proj(0,2)/(0,3) first (their K splits gate tiles 8-15);
                # biasq on DVE and the eq2q exps dead last so neither parks
                # in ACT's depth-0 queue ahead of this block's own exps.
                # proj(1,0) goes late too: its pps wait on head-1 weights
                # and must not crowd the 4-deep PE wait queue early.
                # sequential (not round-robin): block 2's bias quarter must
                # be emitted before tile 8's exp, block 3's before tile 12
                seq = []
                for psbk in range(2, SB):
                    fs = proj_fillers(0, psbk)
                    fs[-2] = (lambda f=fs[-2]: f(no_act=True))
                    late.append(fs.pop())
                    seq += fs
                lanes.append(seq)
                if nxt < N_HEADS_BUILD:
                    late = proj_fillers(nxt, sbk) + late
            elif nxt < N_HEADS_BUILD:
                lanes += [proj_fillers(nxt, sbk)]
            if sbk > 0:
                lanes.insert(1, wo_fillers(h, sbk - 1))
            elif h > 0:
                lanes.insert(1, wo_fillers(h - 1, 3))
            fillers = []
            while any(lanes):
                for ln in lanes:
                    if ln:
                        fillers.append(ln.pop(0))
            fillers += late
            emit_main(h, sbk, fillers)
        if h == N_HEADS_BUILD - 1:
            emit_wo(h, 3)

        if h > 0:
            st_h.pop(h - 1, None)

    if dbg_ext:
        nc.sync.dma_start(dbg_ext['part'][:], acc[:])

    if N_HEADS_BUILD == 0:
        for q in range(4):
            nc.sync.dma_start(
                bounce_view[q][:, :, :],
                acc[:, q * 4 * E:(q + 1) * 4 * E]
                .rearrange("p (t e) -> p t e", e=E))

    # ============ AllReduce over batch pair + store ============
    # four quarters; the last quarter in halves so its store overlaps the
    # second half's bounce write
    QS = S // 4
    pieces = [(q * QS, QS) for q in range(3)] + \
             [(3 * QS, QS // 2), (3 * QS + QS // 2, QS // 2)]
    for pi, (r0, n) in enumerate(pieces):
        osl = out_ext[r0:r0 + n, :]
        q, b0 = divmod(r0, QS)
        if NO_COLL:
            nc.sync.dma_start(osl, bounce_in[q][b0:b0 + n, :])
        else:
            bo = dram.tile([n, E], F32, name=f"bounce_out{pi}",
                           tag=f"bout{pi}", bufs=1)
            nc.gpsimd.collective_compute(
                "AllReduce", OP.add,
                replica_groups=[[0, 1], [2, 3], [4, 5], [6, 7]],
                ins=[bounce_in[q][b0:b0 + n, :].opt()],
                outs=[bo.opt()],
            )
            nc.sync.dma_start(osl, bo[:, :])


# ================= host side =================

def prep_inputs(x, ln_scale, W_q, W_k, W_v, W_o, gamma):
    """Build per-core input maps."""
    x = np.asarray(x, np.float32)
    ln_scale = np.asarray(ln_scale, np.float32)
    W_q = np.asarray(W_q, np.float32)
    W_k = np.asarray(W_k, np.float32)
    W_v = np.asarray(W_v, np.float32)
    W_o = np.asarray(W_o, np.float32)
    gamma = np.asarray(gamma, np.float32).reshape(H)

    in_maps = []
    for c in range(N_CORES):
        b = c // 2
        h0 = HL * (c % 2)
        hs = list(range(h0, h0 + HL))
        g = gamma[hs]
        # 2^WSH folded into W_q/W_k (undone by the exp's scale=2^-2*WSH) so
        # the fp8 hi/lo split of K'/Q' sits well inside e4m3's normal range
        s2g = (np.sqrt(2.0 * g) * (2.0 ** WSH)).astype(np.float32)
        wq = (W_q[hs] * ln_scale[None, :, None] * s2g[:, None, None])
        wk = (W_k[hs] * ln_scale[None, :, None] * s2g[:, None, None])

        def _lay(w):   # [HL, E_in(=EC*128), E] -> [HL, 128, EC*E]
            return np.ascontiguousarray(
                w.reshape(HL, EC, 128, E).transpose(0, 2, 1, 3).reshape(HL, 128, EC * E))
        wq = _lay(wq)
        wk = _lay(wk)
        # V folded into W_o: out_h = (scores @ xn) @ (Wv_h @ Wo_h)
        wo = _lay(np.stack([
            (W_v[h] * ln_scale[:, None]).astype(np.float64)
            @ W_o[:, 256 * h:256 * (h + 1)].T.astype(np.float64)
            for h in hs]).astype(np.float32))
        in_maps.append({
            "x": np.ascontiguousarray(x[b]),
            "wq": np.ascontiguousarray(wq),
            "wk": np.ascontiguousarray(wk),
            "wo": np.ascontiguousarray(wo),
        })
    return in_maps


def assemble_output(results):
    out = np.empty((B, S, E), np.float32)
    for b in range(B):
        out[b] = results[2 * b]["out"]
    return out


_NC_CACHE = {}


def _get_nc():
    if 'nc' not in _NC_CACHE:
        _NC_CACHE['nc'] = build_kernel(R=1, debug=False)
    return _NC_CACHE['nc']


def kernel(x, e=None, p=None, ln_scale=None, W_q=None, W_k=None, W_v=None,
           W_o=None, gamma=None, **_unused):
    """Full-input entry point. e and p are unused by the reference network
    (use_ppe=False config); they are accepted and ignored."""
    in_maps = prep_inputs(x, ln_scale, W_q, W_k, W_v, W_o, gamma)
    nc = _get_nc()
    res = run_bass_kernel_spmd(nc, in_maps, core_ids=list(range(N_CORES)))
    return assemble_output(res.results)

